# revision 42
# baseline (speedup 1.0000x reference)
"""Trainium2 Bass kernel for nn_Encoder_51582557225690 (8-core tensor parallel).

Design: 8-way tensor parallelism (2 attention heads + 256 MLP ff cols per
core). The residual stream lives in SBUF as bf16, feature-major, split into
three token chunks (hidden x2, beacon+forget) so collectives pipeline
against compute. The rms-norm scale is folded into projection outputs
(scaled cos/sin tables for q/k, per-partition tensor_scalar for v,
broadcast mul for MLP gate/up), so matmuls read the residual directly with
no normalized-activation materialization. rsqrt/reciprocal run as
exp(-a*ln(x)) on the Scalar engine (DVE reciprocal is ~5x slower).

Attention output uses an AllGather of the per-core head outputs (0.3-0.6MB)
instead of an AllReduce of the projected delta (2.6MB); every core then
applies the full Wo locally - the projection compute lands exactly in the
window that used to be an AR stall. The MLP keeps chunked AllReduces, each
hidden under the next chunk's compute (bf chunk runs first since its
collective lands first). Only 7 of 8 layers are computed (the reference
records states *entering* each layer); layer 6 computes bf columns only.
"""
import sys
import os

sys.path.insert(0, '/opt/trn_rl_repo')

import numpy as np
import ml_dtypes

import concourse.bass as bass
import concourse.tile as tile
from concourse import mybir
from concourse import bass2jax

BF16 = ml_dtypes.bfloat16
DT32 = mybir.dt.float32
DT16 = mybir.dt.bfloat16
DT8 = mybir.dt.float8e4

AR8 = False           # fp8e4 all-reduce payload — NRT exec-unit crash, keep off
AR_SCALE = 8.0        # host-side scale on Wo/Wd; undone in the delta add

# model dims
L, D, H, HD, F, V, S, M = 8, 1024, 16, 64, 2048, 32000, 1024, 128
NL = 7                  # computed layers (layer 7 is dead)
T = S + 2 * M           # 1280 residual tokens
KV = M + T              # 1408 kv tokens (mem + hidden + beacon + forget)
NC = 8                  # cores
EPS = 1e-5
NEG = -240.0            # additive mask; exp(NEG/8) ~ 9e-14

# per-core shard sizes
DC = D // NC            # 128 head-cols per core (2 heads)
FC = F // NC            # 256 ff-cols per core
NDT = D // 128          # 8 D-tiles
NFT = FC // 128         # 2 f-tiles per core

# weight blob layout (free elems per partition, bf16)
_SEGS_A = ['wq', 'wk', 'wv', 'wbq', 'wbk', 'wbv', 'wfq', 'wfk', 'wfv',
           'wmk', 'wmv', 'mem']
OFF_A = {k: i * 1024 for i, k in enumerate(_SEGS_A)}
WA = len(_SEGS_A) * 1024                      # 12288
OFF_B = {'wg': 0, 'wu': 2048, 'wd': 4096}
WB = 6144
WO = NDT * NDT * 128                          # full Wo, 8x8 tile grid: 8192

# token chunks: c0, c1 hidden halves; c2 = beacon+forget
CHUNKS = [(0, 512), (512, 1024), (1024, 1280)]
# q/k projection column groups: (start, end, weight-prefix)
QK_GROUPS = [(0, 512, 'w'), (512, 1024, 'w'), (1024, 1152, 'wb'), (1152, 1280, 'wf')]
QCH = [(0, 512), (512, 1024), (1024, 1280)]
# allowed kv tiles per q-chunk: list of (kt, mask_idx or None)
ATTN_BLOCKS = {
    0: [(0, None), (1, 0), (2, 1), (3, 2), (4, 3)],
    1: [(0, None), (1, None), (2, None), (3, None), (4, None),
        (5, 0), (6, 1), (7, 2), (8, 3)],
    2: [(0, None), (1, None), (2, None), (3, None), (4, None),
        (5, None), (6, None), (7, None), (8, None), (9, 4), (10, 5)],
}
MASK_BASE = {0: 0, 1: 512, 2: 1024, 3: 1536, 4: 2048, 5: 2304}
MASK_W = {0: 512, 1: 512, 2: 512, 3: 512, 4: 256, 5: 256}


# ---------------------------------------------------------------- host prep

def _to_bf16(a):
    return np.asarray(a, BF16)


def _pack_col_shard(Wl, c, ncols):
    Wc = Wl[:, c * ncols:(c + 1) * ncols]
    return Wc.reshape(NDT, 128, ncols).transpose(1, 0, 2).reshape(128, NDT * ncols)


def build_host_inputs(input_ids, memory, beacon, forget, embed, ln1, ln2,
                      Wq, Wk, Wv, Wo, mWk, mWv, bWq, bWk, bWv,
                      fWq, fWk, fWv, Wg, Wu, Wd):
    """Returns (shared_inputs_dict, per_core_wblobs[8])."""
    ids = np.asarray(input_ids).reshape(-1)
    hidden = np.asarray(embed)[ids]                     # [S, D] f32
    cat0 = np.concatenate([hidden,
                           np.asarray(beacon).reshape(M, D),
                           np.asarray(forget).reshape(M, D)], axis=0)  # [T, D]
    catT = np.ascontiguousarray(cat0.T)                 # [D, T] f32
    cat0_in = _to_bf16(catT.reshape(NDT, 128, T))

    # rope tables in kv layout
    pos = np.arange(KV)
    pos = np.where(pos >= T, pos - M, pos)              # forget keys share bcn pos
    inv = 1.0 / (10000.0 ** (np.arange(0, HD, 2, dtype=np.float64) / HD))  # [32]
    ang = pos[:, None] * inv[None, :]                   # [KV, 32]
    c32 = np.cos(ang).astype(np.float32)                # [KV, 32]
    s32 = np.sin(ang).astype(np.float32)
    cos64 = np.concatenate([c32, c32], axis=1)          # [KV, 64]
    sinp64 = np.concatenate([s32, -s32], axis=1)        # rows 0-31:+s, 32-63:-s
    cosT = np.concatenate([cos64, cos64], axis=1).T     # [128, KV]
    sinpT = np.concatenate([sinp64, sinp64], axis=1).T  # [128, KV]

    # masks
    kk = np.arange(128)[:, None]
    q5 = np.arange(512)[None, :]
    stair = [np.where(q5 >= off + kk, 0.0, NEG).astype(np.float32)
             for off in (0, 128, 256, 384)]
    q2 = np.arange(256)[None, :]
    c1 = np.where((q2 < 128) & (q2 >= kk), 0.0, NEG).astype(np.float32)
    c2 = np.where((q2 >= 128) & (q2 - 128 >= kk), 0.0, NEG).astype(np.float32)
    masks = np.concatenate(stair + [c1, c2], axis=1)    # [128, 2560]

    shared = {
        'cat0': cat0_in,
        'cos': _to_bf16(cosT),
        'sinp': _to_bf16(sinpT),
        'masks': _to_bf16(masks),
    }

    ln1 = np.asarray(ln1)[:, :, None]                   # [L, D, 1]
    ln2 = np.asarray(ln2)[:, :, None]
    mem = np.asarray(memory)

    # full Wo per layer, packed as 8x8 grid of [128,128] tiles:
    # woseg[:, (hc*NDT+dt)*128 : ...] = Wo[hc*128:(hc+1)*128, dt*128:(dt+1)*128]
    wo_full = []
    for l in range(NL):
        Wl = np.asarray(Wo)[l]
        wo_full.append(_to_bf16(
            Wl.reshape(NDT, 128, NDT, 128).transpose(1, 0, 2, 3)
              .reshape(128, WO)))

    blobs = []
    for c in range(NC):
        per_layer = []
        for l in range(NL):
            segs = np.zeros((128, WA + WB + WO), dtype=BF16)
            for key, W in (('wq', Wq), ('wk', Wk), ('wv', Wv),
                           ('wbq', bWq), ('wbk', bWk), ('wbv', bWv),
                           ('wfq', fWq), ('wfk', fWk), ('wfv', fWv)):
                Wl = np.asarray(W)[l] * ln1[l]
                segs[:, OFF_A[key]:OFF_A[key] + 1024] = \
                    _to_bf16(_pack_col_shard(Wl, c, DC))
            for key, W in (('wmk', mWk), ('wmv', mWv)):
                Wl = np.asarray(W)[l]                   # memory is NOT normed
                segs[:, OFF_A[key]:OFF_A[key] + 1024] = \
                    _to_bf16(_pack_col_shard(Wl, c, DC))
            mT = mem[l].T                                # [D, M]
            segs[:, OFF_A['mem']:OFF_A['mem'] + 1024] = _to_bf16(
                mT.reshape(NDT, 128, M).transpose(1, 0, 2).reshape(128, NDT * M))
            for key, W in (('wg', Wg), ('wu', Wu)):
                Wl = np.asarray(W)[l] * ln2[l]
                segs[:, WA + OFF_B[key]:WA + OFF_B[key] + 2048] = \
                    _to_bf16(_pack_col_shard(Wl, c, FC))
            Wdc = np.asarray(Wd)[l][c * FC:(c + 1) * FC, :] * \
                (AR_SCALE if AR8 else 1.0)                       # [256, 1024]
            wdseg = Wdc.reshape(NFT, 128, NDT, 128).transpose(1, 0, 2, 3) \
                       .reshape(128, NFT * NDT * 128)
            segs[:, WA + OFF_B['wd']:WA + OFF_B['wd'] + 2048] = _to_bf16(wdseg)
            segs[:, WA + WB:] = wo_full[l]
            per_layer.append(segs)
        blobs.append(np.stack(per_layer))                # [NL, 128, WA+WB]
    return shared, blobs


def finalize_output(records, memory, beacon, forget):
    """records: [NL, NDT, 128, 256] bf16 (catT bf cols AFTER each of the 7
    computed layers). Output: [L, M, D] f32."""
    memory = np.asarray(memory, np.float64)
    inj = np.empty((L, M, D), np.float64)
    fg = np.empty((L, M, D), np.float64)
    inj[0] = np.asarray(beacon, np.float64).reshape(M, D)
    fg[0] = np.asarray(forget, np.float64).reshape(M, D)
    for l in range(1, L):
        rec = np.asarray(records[l - 1], np.float64)     # [NDT, 128, 256]
        full = rec.reshape(D, 2 * M)                     # [D, 256]
        inj[l] = full[:, :M].T
        fg[l] = full[:, M:].T
    g = 1.0 / (1.0 + np.exp(-fg))
    out = memory * g + inj * (1.0 - g)
    return out.astype(np.float32)


# ---------------------------------------------------------------- bass build

def split_multiwaits(nc):
    """This walrus build allows only 1 sem wait per instruction; hoist
    extras onto preceding same-engine NOPs (sequential waits == AND)."""
    ctr = 0
    for fn in nc.m.functions:
        for bb in fn.blocks:
            plan = {}
            for idx, ins in enumerate(bb.instructions):
                si = ins.sync_info
                if si is not None and si.on_wait and len(si.on_wait) > 1:
                    waits = list(si.on_wait)
                    nops = []
                    for w in waits[:-1]:
                        ctr += 1
                        nop = mybir.InstNoOp(name=f"I-mwfix-{ctr}", ins=[], outs=[])
                        nop.engine = ins.engine
                        nop.sync_info = mybir.SyncInfo(on_wait=[w], on_update=[])
                        nops.append(nop)
                    del si.on_wait[:-1]
                    plan[idx] = nops
            if plan:
                newlist = []
                for idx, ins in enumerate(bb.instructions):
                    if idx in plan:
                        newlist.extend(plan[idx])
                    newlist.append(ins)
                bb.instructions[:] = newlist
    return nc


def build_nc(n_layers=NL, no_coll=False, shared_out=True, **_unused):
    AF = mybir.ActivationFunctionType
    nc = bass.Bass()
    cat0 = nc.dram_tensor("cat0", [NDT, 128, T], DT16, kind="ExternalInput")
    wblob = nc.dram_tensor("wblob", [NL, 128, WA + WB + WO], DT16,
                           kind="ExternalInput")
    cos_in = nc.dram_tensor("cos", [128, KV], DT16, kind="ExternalInput")
    sinp_in = nc.dram_tensor("sinp", [128, KV], DT16, kind="ExternalInput")
    masks_in = nc.dram_tensor("masks", [128, 2560], DT16, kind="ExternalInput")
    records = nc.dram_tensor("records", [NL, NDT, 128, 2 * M], DT16,
                             kind="ExternalOutput")
    RG = [list(range(NC))]

    from contextlib import ExitStack
    with tile.TileContext(nc) as tc, ExitStack() as ctx:
        ep = ctx.enter_context
        constp = ep(tc.tile_pool(name="const", bufs=1))
        catp = ep(tc.tile_pool(name="cat", bufs=1))
        wap = ep(tc.tile_pool(name="wa", bufs=2))
        wbp = ep(tc.tile_pool(name="wb", bufs=2))
        wop = ep(tc.tile_pool(name="wo", bufs=1))
        ogp = ep(tc.tile_pool(name="og", bufs=16))
        qkp = ep(tc.tile_pool(name="qk", bufs=1))
        vp = ep(tc.tile_pool(name="vp", bufs=1))
        probsp = ep(tc.tile_pool(name="probs", bufs=3))
        op_ = ep(tc.tile_pool(name="op", bufs=1))
        hp = ep(tc.tile_pool(name="hp", bufs=1))
        gp = ep(tc.tile_pool(name="gp", bufs=1))
        csp = ep(tc.tile_pool(name="cs", bufs=2))
        stagep = ep(tc.tile_pool(name="stage", bufs=2))
        deltap = ep(tc.tile_pool(name="delta", bufs=2))
        rowsp = ep(tc.tile_pool(name="rows", bufs=3))
        bcastp = ep(tc.tile_pool(name="bcast", bufs=2))
        sqp = ep(tc.tile_pool(name="sq", bufs=2))
        # PSUM: mm 3 + av 2 + ssq 1 + bc 2 = 8 banks
        psMM = ep(tc.tile_pool(name="psMM", bufs=4, space="PSUM"))
        psAV = ep(tc.tile_pool(name="psAV", bufs=2, space="PSUM"))
        psB = ep(tc.tile_pool(name="psB", bufs=2, space="PSUM"))
        dram = ep(tc.tile_pool(name="dram", bufs=1, space="DRAM"))

        # ---------------- constants
        cos_t = constp.tile([128, KV], DT16)
        nc.sync.dma_start(out=cos_t[:], in_=cos_in[:, :])
        sinp_t = constp.tile([128, KV], DT16)
        nc.sync.dma_start(out=sinp_t[:], in_=sinp_in[:, :])
        mask_t = constp.tile([128, 2560], DT16)
        nc.sync.dma_start(out=mask_t[:], in_=masks_in[:, :])
        ones_t = constp.tile([128, 1], DT16)
        nc.any.memset(ones_t[:], 1.0)
        onesb = constp.tile([1, 128], DT16)
        nc.any.memset(onesb[:], 1.0)
        eps_t = constp.tile([128, 1], DT32)
        nc.any.memset(eps_t[:], EPS)

        # residual, bf16, split per chunk: catc[j] [128, NDT, w]
        catc = []
        for j, (c0, c1) in enumerate(CHUNKS):
            w = c1 - c0
            t_ = catp.tile([128, NDT, w], DT16, tag=f"cat{j}", name=f"cat{j}")
            for dt in range(NDT):
                nc.sync.dma_start(out=t_[:, dt, :], in_=cat0[dt, :, c0:c1])
            catc.append(t_)

        # DRAM bounce buffers: merged hidden (A=[0,1024)) + bf (B) payloads.
        # Layout [128, NDT, w] matches the SBUF staging tiles so each
        # chunk moves in ONE dma; the AR only needs a consistent flat view.
        ARDT = DT8 if AR8 else DT16
        def mk_bounce(tag, w):
            i = dram.tile([128, NDT, w], ARDT, tag=tag + "i", name=tag + "i")
            if shared_out:
                o = nc.dram_tensor(tag + "o", [128, NDT, w], ARDT,
                                   addr_space="Shared")
            else:
                o = dram.tile([128, NDT, w], ARDT, tag=tag + "o", name=tag + "o")
            return i, o

        b2c0 = mk_bounce("b2c0", 512)
        b2c1 = mk_bounce("b2c1", 512)
        bB2 = mk_bounce("bB2", 256)

        # AllGather buffers for attention head outputs
        def mk_ag(tag, w):
            i = dram.tile([128, w], DT16, tag=tag + "i", name=tag + "i")
            o = nc.dram_tensor(tag + "o", [NC, 128, w], DT16,
                               addr_space="Shared")
            return i, o

        agA = mk_ag("agA", 1024)
        agB = mk_ag("agB", 256)

        def load_weights(l):
            wA = wap.tile([128, WA], DT16, tag="wA")
            for j in range(4):
                w0 = j * (WA // 4)
                nc.sync.dma_start(out=wA[:, w0:w0 + WA // 4],
                                  in_=wblob[l, :, w0:w0 + WA // 4])
            wB = wbp.tile([128, WB], DT16, tag="wB")
            for j in range(2):
                w0 = j * (WB // 2)
                nc.sync.dma_start(out=wB[:, w0:w0 + WB // 2],
                                  in_=wblob[l, :, WA + w0:WA + w0 + WB // 2])
            wO = wop.tile([128, WO], DT16, tag="wO")
            for j in range(2):
                w0 = j * (WO // 2)
                nc.sync.dma_start(out=wO[:, w0:w0 + WO // 2],
                                  in_=wblob[l, :, WA + WB + w0:WA + WB + w0 + WO // 2])
            return wA, wB, wO

        def stats(j, with_rope):
            """rms stats for chunk j off current catc[j].
            Returns (bc PSUM [128,w] f32, rowc [1,w] bf16) and, if with_rope,
            (cs, ss) scaled rope tables bf16 [128,w]."""
            c0, c1 = CHUNKS[j]
            w = c1 - c0
            ssq = psB.tile([1, 512], DT32, tag="bc")
            for dt in range(NDT):
                sq = sqp.tile([128, 512], DT16, tag="sq")
                if dt % 2 == 0:
                    nc.scalar.square(sq[:, :w], catc[j][:, dt, :])
                else:
                    nc.vector.tensor_mul(sq[:, :w], catc[j][:, dt, :],
                                         catc[j][:, dt, :])
                nc.tensor.matmul(ssq[:, :w], ones_t[:], sq[:, :w],
                                 start=(dt == 0), stop=(dt == NDT - 1))
            rowa = rowsp.tile([1, 512], DT32, tag="rowa")
            nc.scalar.activation(rowa[:, :w], ssq[:, :w], AF.Ln,
                                 bias=eps_t[0:1, :], scale=1.0 / D)
            rowc = rowsp.tile([1, 512], DT16, tag="rowc")
            nc.scalar.activation(rowc[:, :w], rowa[:, :w], AF.Exp, scale=-0.5)
            bc = psB.tile([128, 512], DT32, tag="bc")
            nc.tensor.matmul(bc[:, :w], onesb[:], rowc[:, :w],
                             start=True, stop=True)
            if not with_rope:
                return bc, rowc, None, None
            cs = csp.tile([128, 512], DT16, tag="cs")
            nc.vector.tensor_mul(cs[:, :w], cos_t[:, M + c0:M + c1], bc[:, :w])
            ss = csp.tile([128, 512], DT16, tag="ss")
            nc.vector.tensor_mul(ss[:, :w], sinp_t[:, M + c0:M + c1], bc[:, :w])
            return bc, rowc, cs, ss

        def rope_scaled(dst, dst0, psrc, w, cs, ss):
            """dst[:, dst0:dst0+w] = rope(psrc)*scale via scaled tables.
            psrc is copied to bf16 first so the DVE muls run in 2x mode."""
            qb = sqp.tile([128, 512], DT16, tag="qb")
            nc.scalar.copy(qb[:, :w], psrc[:, :w])
            b = sqp.tile([128, 512], DT16, tag="ropeB")
            nc.vector.tensor_mul(dst[:, dst0:dst0 + w], qb[:, :w], cs[:, :w])
            for hb in (0, 64):
                nc.vector.tensor_mul(
                    b[hb + 0:hb + 32, :w], qb[hb + 32:hb + 64, :w],
                    ss[hb + 32:hb + 64, :w])
                nc.vector.tensor_mul(
                    b[hb + 32:hb + 64, :w], qb[hb + 0:hb + 32, :w],
                    ss[hb + 0:hb + 32, :w])
            nc.vector.tensor_add(dst[:, dst0:dst0 + w],
                                 dst[:, dst0:dst0 + w], b[:, :w])

        def rope_raw(dst, dst0, psrc, w, tab0):
            """unscaled rope (memory keys) using raw cos/sinp tables."""
            qb = sqp.tile([128, 512], DT16, tag="qb")
            nc.scalar.copy(qb[:, :w], psrc[:, :w])
            b = sqp.tile([128, 512], DT16, tag="ropeB")
            nc.vector.tensor_mul(dst[:, dst0:dst0 + w], qb[:, :w],
                                 cos_t[:, tab0:tab0 + w])
            for hb in (0, 64):
                nc.vector.tensor_mul(
                    b[hb + 0:hb + 32, :w], qb[hb + 32:hb + 64, :w],
                    sinp_t[hb + 32:hb + 64, tab0:tab0 + w])
                nc.vector.tensor_mul(
                    b[hb + 32:hb + 64, :w], qb[hb + 0:hb + 32, :w],
                    sinp_t[hb + 0:hb + 32, tab0:tab0 + w])
            nc.vector.tensor_add(dst[:, dst0:dst0 + w],
                                 dst[:, dst0:dst0 + w], b[:, :w])

        # ---------------- layers
        for l in range(n_layers):
            last = (l == NL - 1)
            wA, wB, wO = load_weights(l)

            def wseg(key, dt):
                o = OFF_A[key] + dt * 128
                return wA[:, o:o + 128]

            def catslice(dt, g0, g1):
                """catT bf16 slice covering token cols [g0,g1) (must lie
                within one chunk)."""
                for j, (c0, c1) in enumerate(CHUNKS):
                    if g0 >= c0 and g1 <= c1:
                        return catc[j][:, dt, g0 - c0:g1 - c0]
                raise AssertionError((g0, g1))

            # ---- phase 1: memory kv first (depends only on weights, so
            # it fills the tail of the previous layer's last collective),
            # then stats + qkv for all chunks
            qTr = qkp.tile([128, T], DT16, tag="q")
            kTr = qkp.tile([128, KV], DT16, tag="k")
            # memory keys (kv cols 0:128)
            pk = psMM.tile([128, 512], DT32, tag="mm")
            for dt in range(NDT):
                nc.tensor.matmul(pk[:, :M], wseg('wmk', dt),
                                 wA[:, OFF_A['mem'] + dt * 128:
                                     OFF_A['mem'] + (dt + 1) * 128],
                                 start=(dt == 0), stop=(dt == NDT - 1))
            rope_raw(kTr, 0, pk, M, 0)
            # memory values -> v_aug tile 0 (unscaled)
            v_aug = vp.tile([128, 11, 130], DT16, tag="v")
            pv = psMM.tile([128, 512], DT32, tag="mm")
            for dt in range(NDT):
                nc.tensor.matmul(
                    pv[:, :128],
                    wA[:, OFF_A['mem'] + dt * 128:OFF_A['mem'] + (dt + 1) * 128],
                    wseg('wmv', dt),
                    start=(dt == 0), stop=(dt == NDT - 1))
            dstv = v_aug[:, 0, :].rearrange("p (g c) -> p g c", g=2)
            nc.any.tensor_copy(dstv[:, :, 0:64],
                               pv[:, :128].rearrange("p (g c) -> p g c", g=2))
            nc.any.memset(dstv[:, :, 64:65], 1.0)

            st_info = [stats(j, with_rope=True) for j in range(3)]

            # q/k for each projection group, using chunk-scaled tables
            for (g0, g1, pre) in QK_GROUPS:
                w = g1 - g0
                j = 0 if g1 <= 512 else (1 if g1 <= 1024 else 2)
                _, _, cs, ss = st_info[j]
                coff = g0 - CHUNKS[j][0]
                if not (last and g1 <= S):  # last layer: only bf queries
                    pq = psMM.tile([128, 512], DT32, tag="mm")
                    for dt in range(NDT):
                        nc.tensor.matmul(pq[:, :w], wseg(pre + 'q', dt),
                                         catslice(dt, g0, g1),
                                         start=(dt == 0), stop=(dt == NDT - 1))
                    rope_scaled(qTr, g0, pq, w, cs[:, coff:coff + w],
                                ss[:, coff:coff + w])
                pk = psMM.tile([128, 512], DT32, tag="mm")
                for dt in range(NDT):
                    nc.tensor.matmul(pk[:, :w], wseg(pre + 'k', dt),
                                     catslice(dt, g0, g1),
                                     start=(dt == 0), stop=(dt == NDT - 1))
                rope_scaled(kTr, M + g0, pk, w, cs[:, coff:coff + w],
                            ss[:, coff:coff + w])

            # v projection tiles 1..10, scaled per token (partition) norm
            for kt in range(1, 11):
                ct = kt - 1
                g0, g1 = ct * 128, (ct + 1) * 128
                j = 0 if g1 <= 512 else (1 if g1 <= 1024 else 2)
                _, rowc, _, _ = st_info[j]
                coff = g0 - CHUNKS[j][0]
                wkey = 'wv' if ct < 8 else ('wbv' if ct == 8 else 'wfv')
                pv = psMM.tile([128, 512], DT32, tag="mm")
                for dt in range(NDT):
                    nc.tensor.matmul(
                        pv[:, :128],
                        catslice(dt, g0, g1),
                        wseg(wkey, dt),
                        start=(dt == 0), stop=(dt == NDT - 1))
                # sT: column layout of the token scales [128,1]
                sT = psB.tile([128, 512], DT32, tag="bc")
                nc.tensor.matmul(sT[:, 0:1], rowc[0:1, coff:coff + 128],
                                 onesb[0:1, 0:1], start=True, stop=True)
                dstv = v_aug[:, kt, :].rearrange("p (g c) -> p g c", g=2)
                nc.vector.tensor_scalar_mul(
                    dstv[:, :, 0:64],
                    pv[:, :128].rearrange("p (g c) -> p g c", g=2),
                    sT[:, 0:1])
                nc.any.memset(dstv[:, :, 64:65], 1.0)

            # ---- attention for one q-chunk into the shared oT tile
            oT = op_.tile([128, T], DT16, tag="oT", name=f"oT{l}")

            def attn(qc):
                q0, q1 = QCH[qc]
                w = q1 - q0
                for h in (0, 1):
                    hb = h * 64
                    pav = psAV.tile([128, 512], DT32, tag="av")
                    blocks = ATTN_BLOCKS[qc]
                    for bi, (kt, mi) in enumerate(blocks):
                        ps = psMM.tile([128, 512], DT32, tag="mm")
                        nc.tensor.matmul(
                            ps[:, :w],
                            kTr[hb:hb + 64, kt * 128:(kt + 1) * 128],
                            qTr[hb:hb + 64, q0:q1],
                            start=True, stop=True)
                        if mi is not None:
                            mb = MASK_BASE[mi]
                            nc.any.tensor_add(ps[:, :w], ps[:, :w],
                                              mask_t[:, mb:mb + w])
                        pr = probsp.tile([128, 512], DT16, tag="pr")
                        nc.scalar.activation(pr[:, :w], ps[:, :w],
                                             AF.Exp, scale=0.125)
                        nc.tensor.matmul(
                            pav[0:65, :w],
                            v_aug[:, kt, :].rearrange(
                                "p (g c) -> p g c", g=2)[:, h, :],
                            pr[:, :w],
                            start=(bi == 0), stop=(bi == len(blocks) - 1))
                    # normalize rows 0:64 by row 64: 1/x = exp(-ln(x))
                    rsum = rowsp.tile([1, 512], DT32, tag="rs")
                    nc.scalar.activation(rsum[:, :w], pav[64:65, :w], AF.Ln)
                    rsumc = rowsp.tile([1, 512], DT16, tag="rsc")
                    nc.scalar.activation(rsumc[:, :w], rsum[:, :w],
                                         AF.Exp, scale=-1.0)
                    bcp = psB.tile([128, 512], DT32, tag="bc")
                    nc.tensor.matmul(bcp[0:64, :w], onesb[:, 0:64],
                                     rsumc[:, :w], start=True, stop=True)
                    bcs = bcastp.tile([64, 512], DT16, tag="bcs")
                    nc.any.tensor_copy(bcs[:, :w], bcp[0:64, :w])
                    nc.vector.tensor_mul(oT[hb:hb + 64, q0:q1],
                                         pav[0:64, :w], bcs[:, :w])

            def do_ar(bin_, bout_):
                if no_coll:
                    return bin_
                nc.gpsimd.collective_compute(
                    "AllReduce", mybir.AluOpType.add, replica_groups=RG,
                    ins=[bin_[:, :, :].opt()], outs=[bout_[:, :, :].opt()])
                return bout_

            def do_ag(bin_, bout_):
                if no_coll:
                    return bout_
                nc.gpsimd.collective_compute(
                    "AllGather", mybir.AluOpType.bypass, replica_groups=RG,
                    ins=[bin_[:, :].opt()], outs=[bout_[:, :, :].opt()])
                return bout_

            def wo_full(j, agout, aoff):
                """Load gathered head outputs for chunk j and apply the full
                Wo locally; add result into catc[j]. Per-hc tiles + split
                DMA rings so the first matmul starts as soon as its slice
                lands instead of waiting for all 8 loads."""
                c0, c1 = CHUNKS[j]
                w = c1 - c0
                ogs = []
                for hc in range(NC):
                    t = ogp.tile([128, 512], DT16, tag="og")
                    eng = nc.sync if hc % 2 == 0 else nc.scalar
                    eng.dma_start(out=t[:, :w],
                                  in_=agout[hc, :, aoff:aoff + w])
                    ogs.append(t)
                for dt in range(NDT):
                    po = psMM.tile([128, 512], DT32, tag="mm")
                    for hc in range(NC):
                        o = (hc * NDT + dt) * 128
                        nc.tensor.matmul(po[:, :w], wO[:, o:o + 128],
                                         ogs[hc][:, :w],
                                         start=(hc == 0), stop=(hc == NC - 1))
                    nc.vector.tensor_add(catc[j][:, dt, :],
                                         catc[j][:, dt, :], po[:, :w])

            def add_delta(j, bout_):
                c0, c1 = CHUNKS[j]
                w = c1 - c0
                for dt in range(NDT):
                    de = deltap.tile([128, 512], ARDT, tag="de")
                    nc.sync.dma_start(out=de[:, :w], in_=bout_[:, dt, :])
                    if AR8:
                        nc.vector.scalar_tensor_tensor(
                            catc[j][:, dt, :], de[:, :w], 1.0 / AR_SCALE,
                            catc[j][:, dt, :],
                            mybir.AluOpType.mult, mybir.AluOpType.add)
                    else:
                        nc.gpsimd.tensor_add(catc[j][:, dt, :],
                                             catc[j][:, dt, :], de[:, :w])

            def mlp_chunk(j, bounce):
                """rms2 stats + gated MLP for chunk j; stage + dma to bounce."""
                c0, c1 = CHUNKS[j]
                w = c1 - c0
                bc2, _, _, _ = stats(j, with_rope=False)
                bcs2 = bcastp.tile([128, 512], DT16, tag="bcs2")
                nc.any.tensor_copy(bcs2[:, :w], bc2[:, :w])
                hT = []
                for ft in range(NFT):
                    ht = hp.tile([128, 512], DT16, tag=f"h{ft}", name=f"h{ft}_{l}_{j}")
                    pg = psMM.tile([128, 512], DT32, tag="mm")
                    for dt in range(NDT):
                        o = OFF_B['wg'] + dt * FC + ft * 128
                        nc.tensor.matmul(pg[:, :w], wB[:, o:o + 128],
                                         catc[j][:, dt, :],
                                         start=(dt == 0), stop=(dt == NDT - 1))
                    mg = gp.tile([128, 512], DT16, tag="mg")
                    nc.vector.tensor_mul(mg[:, :w], pg[:, :w], bcs2[:, :w])
                    sg = gp.tile([128, 512], DT16, tag="sg")
                    nc.scalar.activation(sg[:, :w], mg[:, :w], AF.Sigmoid)
                    gt = gp.tile([128, 512], DT16, tag="gt")
                    nc.vector.tensor_mul(gt[:, :w], sg[:, :w], mg[:, :w])
                    pu = psMM.tile([128, 512], DT32, tag="mm")
                    for dt in range(NDT):
                        o = OFF_B['wu'] + dt * FC + ft * 128
                        nc.tensor.matmul(pu[:, :w], wB[:, o:o + 128],
                                         catc[j][:, dt, :],
                                         start=(dt == 0), stop=(dt == NDT - 1))
                    mu = gp.tile([128, 512], DT16, tag="mu")
                    nc.vector.tensor_mul(mu[:, :w], pu[:, :w], bcs2[:, :w])
                    nc.vector.tensor_mul(ht[:, :w], gt[:, :w], mu[:, :w])
                    hT.append(ht)
                st3 = stagep.tile([128, NDT, 512], ARDT, tag="st")
                for dt in range(NDT):
                    pd = psMM.tile([128, 512], DT32, tag="mm")
                    for ft in range(NFT):
                        o = OFF_B['wd'] + (ft * NDT + dt) * 128
                        nc.tensor.matmul(pd[:, :w], wB[:, o:o + 128],
                                         hT[ft][:, :w],
                                         start=(ft == 0), stop=(ft == NFT - 1))
                    if dt % 2 == 0:
                        nc.scalar.copy(st3[:, dt, :w], pd[:, :w])
                    else:
                        nc.vector.tensor_copy(st3[:, dt, :w], pd[:, :w])
                nc.sync.dma_start(out=bounce[:, :, :], in_=st3[:, :, :w])

            # ---- phase 2+: attention -> AllGather(oT) -> local full-Wo;
            # MLP per chunk with its AR hidden under the next chunk's work.
            if last:
                attn(2)
                nc.sync.dma_start(out=agB[0][:, :], in_=oT[:, 1024:1280])
                agBo = do_ag(*agB)
                wo_full(2, agBo, 0)
                mlp_chunk(2, bB2[0])
                o2b = do_ar(*bB2)
                add_delta(2, o2b)
            else:
                attn(2)
                nc.sync.dma_start(out=agB[0][:, :], in_=oT[:, 1024:1280])
                agBo = do_ag(*agB)
                attn(0)
                attn(1)
                nc.sync.dma_start(out=agA[0][:, :], in_=oT[:, 0:1024])
                agAo = do_ag(*agA)
                # B tail first (its AG lands first)
                wo_full(2, agBo, 0)
                mlp_chunk(2, bB2[0])
                o2b = do_ar(*bB2)
                wo_full(0, agAo, 0)
                mlp_chunk(0, b2c0[0])
                o2c0 = do_ar(*b2c0)
                wo_full(1, agAo, 512)
                mlp_chunk(1, b2c1[0])
                o2c1 = do_ar(*b2c1)
                add_delta(2, o2b)
                add_delta(0, o2c0)
                add_delta(1, o2c1)
            # record the state entering layer l+1 (bf token columns)
            for dt in range(NDT):
                nc.sync.dma_start(out=records[l, dt, :, :],
                                  in_=catc[2][:, dt, :])
    return nc


# ---------------------------------------------------------------- runner

def make_runner(nc, n_cores=NC):
    import jax
    from jax.sharding import Mesh, PartitionSpec, NamedSharding
    from jax.experimental.shard_map import shard_map
    bass2jax.install_neuronx_cc_hook()
    split_multiwaits(nc)
    partition_name = nc.partition_id_tensor.name if nc.partition_id_tensor else None
    in_names, out_names, out_avals, zero_outs = [], [], [], []
    for alloc in nc.m.functions[0].allocations:
        if not isinstance(alloc, mybir.MemoryLocationSet):
            continue
        name = alloc.memorylocations[0].name
        if alloc.kind == "ExternalInput":
            if name != partition_name:
                in_names.append(name)
        elif alloc.kind == "ExternalOutput":
            out_names.append(name)
            shape = tuple(alloc.tensor_shape)
            dtype = mybir.dt.np(alloc.dtype)
            out_avals.append(jax.core.ShapedArray(shape, dtype))
            zero_outs.append(np.zeros(shape, dtype))
    n_params, n_outs = len(in_names), len(out_avals)
    all_in_names = in_names + out_names
    if partition_name is not None:
        all_in_names = all_in_names + [partition_name]

    def _body(*args):
        operands = list(args)
        if partition_name is not None:
            operands.append(bass2jax.partition_id_tensor())
        outs = bass2jax._bass_exec_p.bind(
            *operands, out_avals=tuple(out_avals), in_names=tuple(all_in_names),
            out_names=tuple(out_names), lowering_input_output_aliases=(),
            sim_require_finite=True, sim_require_nnan=True, nc=nc)
        return tuple(outs)

    devices = jax.devices()[:n_cores]
    mesh = Mesh(np.asarray(devices), ("core",))
    sharding = NamedSharding(mesh, PartitionSpec("core"))
    sharded = jax.jit(
        shard_map(_body, mesh=mesh,
                  in_specs=(PartitionSpec("core"),) * (n_params + n_outs),
                  out_specs=(PartitionSpec("core"),) * n_outs, check_rep=False),
        keep_unused=True)

    def put(in_maps):
        import jax as _jax
        dev_in = []
        for name in in_names:
            cat = np.concatenate([np.asarray(m[name]) for m in in_maps], axis=0)
            dev_in.append(_jax.device_put(cat, sharding))
        for z in zero_outs:
            cat = np.concatenate([z] * n_cores, axis=0)
            dev_in.append(_jax.device_put(cat, sharding))
        return dev_in

    def run_dev(dev_in, reps=1):
        import jax as _jax
        outs = None
        for _ in range(reps):
            outs = sharded(*dev_in)
        _jax.block_until_ready(outs)
        return outs

    def unpack(outs):
        outs = [np.asarray(o) for o in outs]
        res = []
        for c in range(n_cores):
            m = {}
            for i, name in enumerate(out_names):
                sh0 = out_avals[i].shape[0]
                m[name] = outs[i][c * sh0:(c + 1) * sh0]
            res.append(m)
        return res

    return put, run_dev, unpack


_CACHE = {}


def _get_compiled(debug_cat=False):
    key = ('k', debug_cat)
    if key not in _CACHE:
        nc = build_nc(NL, shared_out=True)
        _CACHE[key] = make_runner(nc)
    return _CACHE[key]


def kernel(**inputs):
    shared, blobs = build_host_inputs(**inputs)
    put, run_dev, unpack = _get_compiled()
    in_maps = []
    for c in range(NC):
        m = dict(shared)
        m['wblob'] = blobs[c]
        in_maps.append(m)
    dev_in = put(in_maps)
    outs = run_dev(dev_in)
    res = unpack(outs)
    records = res[0]['records']
    out = finalize_output(records, inputs['memory'], inputs['beacon'],
                          inputs['forget'])
    return out
